# revision 16
# baseline (speedup 1.0000x reference)
"""Trainium2 Bass kernel for ClusteringMMD.

Per graph (batch-sharded 16+16 graphs onto each of 8 cores):
  - host converts the f32 adjacency to fp8e4 bytes directly ({0,1} ->
    {0x00,0x38}) in the [128,4,512] partition-major layout the device
    wants, and folds +32 onto the diagonal (byte 0x60): the device
    tensor is M = A + 32*I (values {0,1,32}, all exact in fp8e4).
    Host also computes deg = rowsum(A); the device only produces the
    masked rowsums S.
  - TensorE: M^2 = M @ M via fp8 DoubleRow matmuls into PSUM (exact:
    M^2 = A^2 + 64*A + 1024*I, entries < 1060, fp32 accumulate; the
    e10m10 pair-sum intermediate stays exact because 1024+1 = 1025
    fits in 11 bits -- this is why the diag constant is 32, not 64).
  - PSUM drain rotates across three engines so no single engine
    bottlenecks: VectorE / GpSimd use tensor_scalar max(x,64) with an
    add-accumulate (S = tri2 + deg + S_OFF); ScalarE uses activation
    Relu(x-64) with accumulate (S' = S - 64*N).  Off-diagonal:
    A^2_ij + 64*A_ij vs the 64 threshold isolates A^2_ij when A_ij=1
    (A^2_ij <= max_deg < 64 when A_ij=0), so the accumulate is exactly
    the masked rowsum tri2_i = sum_j A^2_ij A_ij plus deg_i + 1024 from
    the diagonal.
  - input DMAs ride the sync-engine HWDGE ring (SP is otherwise idle);
    the tiny S outputs go out on the gpsimd SWDGE queue in two chunks.
Host: select per-column S from the right engine's output tile, subtract
the engine-specific offset and deg, then the reference's exact binning
and the tiny [128,100] histogram MMD in f64.

The walrus build in this container rejects instructions carrying more
than one sync wait; _patch_compiler_wait_split() rewrites the BIR JSON
right before compilation, moving excess waits onto same-engine NoOps
inserted immediately before the over-subscribed instruction.
"""

import json
import numpy as np

B = 128
N = 512
BINS = 100
SIGMA = 1.0
N_CORES = 8
PER = B // N_CORES          # graphs per input tensor per core
GP = 2 * PER                # graphs per core (adj_1 shard + adj_2 shard)
P = 128
T = N // P                  # 4 row-blocks

DIAG_C = 32.0               # diagonal fold constant (see module docstring)
MASK_TH = 2.0 * DIAG_C      # mask threshold: M^2 off-diag = A^2 + 64*A
# drain: S_i = sum_j max(M^2_ij, 64) = tri2_i + deg_i + (64*(N-1) + 1024)
S_OFF = MASK_TH * (N - 1) + DIAG_C * DIAG_C
WAIT_CAP = 1                # max sync waits this walrus accepts per inst

FP8_ONE = 0x38              # fp8e4m3 bit pattern of 1.0
FP8_DIAG = 0x60             # fp8e4m3 bit pattern of 32.0

# drain-engine rotation by global tile column (g*T + m), period 16:
# 0 = vector (tensor_scalar max+add), 1 = scalar (Relu(x-64) accum).
# GpSimd cannot read PSUM on TRN2, so only these two engines can drain.
# Vector is slightly cheaper per tile -> 9:7 split.
ENG_PATTERN = (0, 1, 0, 1, 0, 1, 0, 1, 0, 1, 0, 1, 0, 0, 1, 0)

_NC_CACHE = {}


def _engine_of(col):
    return ENG_PATTERN[col % len(ENG_PATTERN)]


def _split_waits(bir_json, cap=WAIT_CAP):
    """Rewrite BIR JSON so no instruction carries more than `cap` sync
    waits; excess waits move to NoOps inserted just before it on the same
    engine (per-engine program order is list order within a block)."""
    m = json.loads(bir_json)
    ctr = 0
    for fn in m.get("functions", []):
        for blk in fn.get("blocks", []):
            out = []
            changed = False
            for ins in blk.get("instructions", []):
                si = ins.get("sync_info")
                waits = (si or {}).get("on_wait") or []
                if len(waits) > cap:
                    changed = True
                    for i in range(0, len(waits) - cap, cap):
                        ctr += 1
                        out.append(
                            {
                                "debug": ins.get("debug", 0),
                                "engine": ins["engine"],
                                "ins": [],
                                "name": f"WSPLIT-{ctr}",
                                "opcode": "NoOp",
                                "outs": [],
                                "text_hint": "wait_split",
                                "sync_info": {
                                    "on_wait": waits[i : i + cap],
                                    "on_update": [],
                                },
                            }
                        )
                    si["on_wait"] = waits[len(waits) - cap :]
                out.append(ins)
            if changed:
                blk["instructions"] = out
    return json.dumps(m).encode()


def _patch_compiler_wait_split():
    import concourse.bass_utils as bu
    import concourse.bass2jax as b2j

    if getattr(bu, "_wait_split_patched", False):
        return
    orig = bu.compile_bir_kernel

    def wrapped(bir_json, tmpdir, neff_name="file.neff"):
        return orig(_split_waits(bir_json), tmpdir, neff_name)

    bu.compile_bir_kernel = wrapped
    b2j.compile_bir_kernel = wrapped
    bu._wait_split_patched = True


def build_nc(gp=GP):
    import concourse.bass as bass
    import concourse.mybir as mybir
    from concourse.tile import TileContext
    from contextlib import ExitStack

    _patch_compiler_wait_split()
    dt = mybir.dt

    nc = bass.Bass(
        "TRN2", target_bir_lowering=False, debug=False, num_devices=N_CORES
    )
    # input pre-permuted + pre-cast on host:
    # a[g, p, t, n] = fp8e4((A_g + 32 I)[t*128 + p, n])
    a = nc.declare_dram_parameter("a", [gp, P, T, N], dt.float8e4, isOutput=False)
    # per-engine S accumulators, partition-major: st_e[p, g*T + m] =
    # S_g[m*128 + p] for the tiles that engine drained (others garbage)
    ot_v = nc.declare_dram_parameter("ot_v", [P, gp * T], dt.float32, isOutput=True)
    ot_a = nc.declare_dram_parameter("ot_a", [P, gp * T], dt.float32, isOutput=True)

    with TileContext(nc) as tc, ExitStack() as ctx:
        pconst = ctx.enter_context(tc.tile_pool(name="const", bufs=1))
        pa8 = ctx.enter_context(tc.tile_pool(name="a8", bufs=14))
        pps = ctx.enter_context(tc.tile_pool(name="ps", bufs=8, space="PSUM"))

        st = [
            pconst.tile([P, gp * T], dt.float32, name=f"st{e}")
            for e in range(2)
        ]
        # per-engine dummy stores for the drain's elementwise output
        # (same-engine reuse costs no cross-engine syncs)
        dum = [
            pconst.tile([P, N], dt.float32, name=f"dum{e}") for e in range(2)
        ]
        # bias AP for the scalar-engine Relu drain (no const AP registered
        # for -64.0 at Bass init)
        neg_th = pconst.tile([P, 1], dt.float32, name="neg_th")
        nc.gpsimd.memset(neg_th[:], -MASK_TH)

        # p-state warmup: keep the PE busy with throwaway matmuls while the
        # first input DMA is in flight, so the clock is ramped before the
        # first real matmul issues
        ones16 = pconst.tile([P, 8], dt.float16, name="ones16")
        nc.gpsimd.memset(ones16[:], 1.0)
        warm = pps.tile([8, 64], dt.float32, name="ps")
        for _ in range(56):
            nc.tensor.matmul(
                warm[0:8, 0:8],
                ones16[:, 0:8],
                ones16[:, 0:8],
                start=True,
                stop=True,
                skip_group_check=True,
            )

        for g in range(gp):
            a8 = pa8.tile([P, T, N], dt.float8e4)
            # alternate HWDGE (sync) / SWDGE (gpsimd) queues for aggregate
            # DMA bandwidth; split graph 0 so the first matmul starts on
            # the half-transfer
            if g == 0:
                nc.sync.dma_start(out=a8[:, 0:2, :], in_=a[g, :, 0:2, :])
                nc.sync.dma_start(out=a8[:, 2:4, :], in_=a[g, :, 2:4, :])
            elif g % 2 == 1:
                # odd graphs (incl. g=1, needed early) on the low-latency
                # HWDGE ring; even graphs on the SWDGE queue
                nc.sync.dma_start(out=a8[:], in_=a[g])
            else:
                nc.gpsimd.dma_start(out=a8[:], in_=a[g])
            for m in range(T):
                ps = pps.tile([P, N], dt.float32)
                for kk in range(T // 2):
                    nc.tensor.matmul(
                        ps[:],
                        a8[:, 2 * kk : 2 * kk + 2, m * P : (m + 1) * P],
                        a8[:, 2 * kk : 2 * kk + 2, :],
                        start=(kk == 0),
                        stop=(kk == T // 2 - 1),
                        perf_mode=mybir.MatmulPerfMode.DoubleRow,
                    )
                col = g * T + m
                e = _engine_of(col)
                acc = st[e][:, col : col + 1]
                if e == 0:
                    nc.vector.tensor_scalar(
                        dum[0][:], ps[:], MASK_TH, 0.0,
                        mybir.AluOpType.max, mybir.AluOpType.add,
                        accum_out=acc,
                    )
                else:
                    nc.scalar.activation(
                        dum[1][:], ps[:],
                        mybir.ActivationFunctionType.Relu,
                        bias=neg_th[:], scale=1.0,
                        accum_out=acc,
                    )
            if (g + 1) % 8 == 0:
                g0 = (g - 7) * T
                g1 = (g + 1) * T
                # chunked output flushes keep the end-of-kernel tail short
                nc.sync.dma_start(out=ot_v[:, g0:g1], in_=st[0][:, g0:g1])
                nc.sync.dma_start(out=ot_a[:, g0:g1], in_=st[1][:, g0:g1])
    return nc


def _get_nc():
    key = GP
    if key not in _NC_CACHE:
        _NC_CACHE[key] = build_nc(key)
    return _NC_CACHE[key]


def _prep(adj):
    """[B, 512, 512] f32 -> (fp8 bytes [B, 128, 4, 512] with the +32
    diagonal fold, deg [B, 512] f32)."""
    b = adj != 0
    deg = b.sum(-1, dtype=np.int32).astype(np.float32)
    u8 = b.astype(np.uint8) * np.uint8(FP8_ONE)
    u8 = np.ascontiguousarray(
        u8.reshape(-1, T, P, N).transpose(0, 2, 1, 3)
    )
    p_idx = np.arange(P)[:, None]
    t_idx = np.arange(T)[None, :]
    u8[:, p_idx, t_idx, t_idx * P + p_idx] = np.uint8(FP8_DIAG)
    return u8, deg


def run_device(adj_1, adj_2, trace=False):
    """Run the bass kernel on 8 cores; returns (tri2, deg) for each input
    tensor as [B, N] f32 arrays, plus the BassKernelResults."""
    import concourse.mybir as mybir
    from concourse.bass_utils import run_bass_kernel_spmd

    f8np = mybir.dt.np(mybir.dt.float8e4)
    nc = _get_nc()
    u8_1, deg_1 = _prep(np.asarray(adj_1))
    u8_2, deg_2 = _prep(np.asarray(adj_2))
    in_maps = []
    for c in range(N_CORES):
        shard = np.concatenate(
            [u8_1[c * PER : (c + 1) * PER], u8_2[c * PER : (c + 1) * PER]],
            axis=0,
        )
        in_maps.append({"a": shard.view(f8np)})
    res = run_bass_kernel_spmd(nc, in_maps, list(range(N_CORES)), trace=trace)

    ncols = GP * T
    cols = np.arange(ncols)
    eng = np.array([_engine_of(c) for c in cols])
    # engine-specific offset: scalar's Relu(x-64) drain is S - 64*N
    off = np.float32(S_OFF) - np.float32(MASK_TH * N) * (eng == 1)

    def unscr(r):
        # pick each column from the engine that drained it, then
        # [128, gp*T] partition-major -> [GP, N]
        stk = np.stack([r["ot_v"], r["ot_a"]])  # [2, P, ncols]
        s = stk[eng, :, cols]                               # [ncols, P]
        s = s - off[:, None]
        return s.reshape(GP, T, P).reshape(GP, N)

    tri = np.stack([unscr(r) for r in res.results])  # [ncores, GP, N] = S-off
    deg = np.concatenate(
        [
            np.stack([deg_1[c * PER : (c + 1) * PER] for c in range(N_CORES)]),
            np.stack([deg_2[c * PER : (c + 1) * PER] for c in range(N_CORES)]),
        ],
        axis=1,
    )  # [ncores, GP, N] matching shard order
    tri = tri - deg
    tri2_1 = tri[:, :PER].reshape(B, N)
    tri2_2 = tri[:, PER:].reshape(B, N)
    d1 = deg[:, :PER].reshape(B, N)
    d2 = deg[:, PER:].reshape(B, N)
    return (tri2_1, d1), (tri2_2, d2), res


def _hist(tri2, deg):
    # bit-exact f32 replication of the reference binning
    tri2 = tri2.astype(np.float32)
    deg = deg.astype(np.float32)
    denom = deg * (deg - np.float32(1.0))
    c = np.where(
        denom > 0,
        tri2 / np.maximum(denom, np.float32(1.0)),
        np.float32(0.0),
    ).astype(np.float32)
    idx = np.clip((c * np.float32(BINS)).astype(np.int32), 0, BINS - 1)
    hist = np.zeros((idx.shape[0], BINS), np.float32)
    np.add.at(hist, (np.arange(idx.shape[0])[:, None], idx), np.float32(1.0))
    return hist


def _mmd(x, y):
    x = x.astype(np.float64)
    y = y.astype(np.float64)

    def kmat(a, b):
        sq = (
            (a * a).sum(-1)[:, None]
            + (b * b).sum(-1)[None, :]
            - 2.0 * (a @ b.T)
        )
        return np.exp(-np.maximum(sq, 0.0) / (2.0 * SIGMA * SIGMA))

    return kmat(x, x).mean() + kmat(y, y).mean() - 2.0 * kmat(x, y).mean()


def kernel(adj_1, adj_2):
    (t1, d1), (t2, d2), _ = run_device(adj_1, adj_2)
    h1 = _hist(t1, d1)
    h2 = _hist(t2, d2)
    return np.float32(_mmd(h1, h2))
